# revision 12
# baseline (speedup 1.0000x reference)
"""Trainium2 Bass kernel for Transformer-XL-style relative-position attention.

Problem (per reference):
  T=512 tokens, B=8 batch, D=512 model dim, H=8 heads, DH=64.
  energy = (q+u)@k^T + (q+v)@rpe^T(rel) ; rpe = sinusoidal(i-j) @ W_pos
  softmax over j (diag masked), out = (attn@v) @ W_out + b_out.

Strategy:
  - Data parallel over batch: core b computes batch element b end-to-end.
    No collectives needed.
  - The (t,t,d) rpe tensor is never materialized. Using
    sin((i-j)f) = sin(if)cos(jf) - cos(if)sin(jf) (and cos analog), the
    BD term factorizes exactly into plain matmuls:
      P^T   = W_pos_h^T @ (q+v)^T            (per head, contraction 64)
      C1    = sin(if).P_sin + cos(if).P_cos  (elementwise, DVE)
      C2    = sin(if).P_cos - cos(if).P_sin
      BD^T  = G^T.T @ [C1;C2]  where G = [cos(jf) | sin(jf)] is constant.
  - Everything is computed in feature-major ("transposed") layout so the
    softmax denominator and attn@v reduce over the PSUM-accumulated j
    (partition) axis via an appended ones-column on V.
  - Diagonal mask applied as one extra matmul adding -BIG on the diagonal.
  - bf16 matmul inputs, fp32 PSUM accumulation.
"""

import os
import sys

sys.path.insert(0, "/opt/trn_rl_repo")

import numpy as np
import ml_dtypes

T, B, D, H = 512, 8, 512, 8
DH = D // H
HALF = D // 2
NT = T // 128          # 4 token tiles
ND = D // 128          # 4 feature tiles
NEG_BIG = -30000.0

BF16 = ml_dtypes.bfloat16

_CACHE = {}


def _patch_tile_drain():
    """walrus in this image rejects >1 sync-waits on one TPB_CTRL drain;
    split the TileContext tail-drain waits across several drains."""
    import concourse.tile as tile
    import concourse.mybir as mybir

    if getattr(tile.TileContext, "_drain_patched", False):
        return

    def _drain_and_barrier(self, tick_clock, wait_clock):
        from concourse.vector_clock import ScopedClock

        nc = self.nc
        drain_inst = nc.sync.drain()
        wait_clock.add_sem_waits(
            drain_inst.ins, ScopedClock({None: tick_clock.global_clock})
        )
        si = drain_inst.ins.sync_info
        waits = list(si.on_wait or [])
        if len(waits) > 1:
            si.on_wait[:] = waits[:1]
            for w in waits[1:]:
                extra = nc.sync.drain()
                extra.ins.sync_info = mybir.SyncInfo(on_wait=[w], on_update=[])

        nc.all_engine_barrier()
        assert self.sems is not None
        popped = nc._tile_sem_poison_stack.pop()
        assert popped is self._sem_poison
        nc.clear_and_free_semaphores(list(self.sems.allocated().values()))
        nc.all_engine_barrier()

    tile.TileContext._drain_and_barrier = _drain_and_barrier
    tile.TileContext._drain_patched = True


def _split_multi_waits(nc, limit=1):
    """This walrus build rejects >limit sync-waits on one instruction;
    hoist extra waits onto same-engine NoOp carriers placed just before."""
    import concourse.mybir as mybir

    ctr = [0]
    for f in nc.m.functions:
        for blk in f.blocks:
            new_insts = []
            for inst in blk.instructions:
                si = inst.sync_info
                waits = list(si.on_wait) if si and si.on_wait else []
                if len(waits) > limit:
                    for i in range(limit, len(waits), limit):
                        ctr[0] += 1
                        nop = mybir.InstNoOp(
                            name=f"waitnop{ctr[0]}", ins=[], outs=[]
                        )
                        nop.engine = inst.engine
                        nop.sync_info = mybir.SyncInfo(
                            on_wait=waits[i : i + limit], on_update=[]
                        )
                        new_insts.append(nop)
                    si.on_wait[:] = waits[:limit]
                new_insts.append(inst)
            blk.instructions[:] = new_insts


def _build():
    import concourse.bass as bass
    import concourse.mybir as mybir
    import concourse.tile as tile

    _patch_tile_drain()

    f32 = mybir.dt.float32
    bf16 = mybir.dt.bfloat16
    AF = mybir.ActivationFunctionType

    nc = bass.Bass("TRN2", target_bir_lowering=True, debug=False, num_devices=B)

    with tile.TileContext(nc) as tc:
        # ---- DRAM parameters -------------------------------------------
        xT_d = nc.dram_tensor("xT", [D, T], bf16, kind="ExternalInput")
        wqkv_d = nc.dram_tensor("wqkv", [D, 3 * D], bf16, kind="ExternalInput")
        wpT_d = nc.dram_tensor("wpT", [D, D], bf16, kind="ExternalInput")
        wout_d = nc.dram_tensor("wout", [D, D], bf16, kind="ExternalInput")
        posu_d = nc.dram_tensor("posu", [D, 1], f32, kind="ExternalInput")
        posv_d = nc.dram_tensor("posv", [D, 1], f32, kind="ExternalInput")
        bout_d = nc.dram_tensor("bout", [D, 1], f32, kind="ExternalInput")
        ltau_d = nc.dram_tensor("ltau", [128, 1], f32, kind="ExternalInput")
        gt_d = nc.dram_tensor("gt", [D, T], bf16, kind="ExternalInput")
        sing_d = nc.dram_tensor("sing", [HALF, T], bf16, kind="ExternalInput")
        cosg_d = nc.dram_tensor("cosg", [HALF, T], bf16, kind="ExternalInput")
        dmask_d = nc.dram_tensor("dmask", [128, 896], bf16, kind="ExternalInput")
        dneg_d = nc.dram_tensor("dneg", [128, 128], bf16, kind="ExternalInput")
        out_d = nc.dram_tensor("out", [D, T], f32, kind="ExternalOutput")

        # ---- static SBUF tiles -----------------------------------------
        with tc.tile_pool(name="static", bufs=1) as sp:
            xT = [sp.tile([128, T], bf16, name=f"xT{i}") for i in range(ND)]
            wqkv = [sp.tile([128, 3 * D], bf16, name=f"wqkv{i}") for i in range(ND)]
            wpT = [sp.tile([128, D], bf16, name=f"wpT{i}") for i in range(ND)]
            wout = [sp.tile([128, D], bf16, name=f"wout{i}") for i in range(ND)]
            gt = [sp.tile([128, T], bf16, name=f"gt{i}") for i in range(ND)]
            sing = [sp.tile([128, T], bf16, name=f"sing{i}") for i in range(2)]
            cosg = [sp.tile([128, T], bf16, name=f"cosg{i}") for i in range(2)]
            posu = [sp.tile([128, 1], f32, name=f"posu{i}") for i in range(ND)]
            posv = [sp.tile([128, 1], f32, name=f"posv{i}") for i in range(ND)]
            bout = [sp.tile([128, 1], f32, name=f"bout{i}") for i in range(ND)]
            dmask = sp.tile([128, 896], bf16, name="dmask")
            dneg = sp.tile([128, 128], bf16, name="dneg")
            ltau = sp.tile([128, 1], f32, name="ltau")
            lnm = sp.tile([128, 1], f32, name="lnm")
            mb = sp.tile([128, 1], f32, name="mb")

            quT = [sp.tile([128, T], bf16, name=f"quT{i}") for i in range(ND)]
            qvT = [sp.tile([128, T], bf16, name=f"qvT{i}") for i in range(ND)]
            kT = [sp.tile([128, T], bf16, name=f"kT{i}") for i in range(ND)]
            vsb = [sp.tile([128, 8 * 128], bf16, name=f"v{i}") for i in range(NT)]
            avn = [sp.tile([128, T], bf16, name=f"avn{i}") for i in range(ND)]
            outsb = [sp.tile([128, T], f32, name=f"out{i}") for i in range(ND)]

            for i in range(ND):
                nc.sync.dma_start(xT[i][:], xT_d[i * 128:(i + 1) * 128, :])
                nc.sync.dma_start(wqkv[i][:], wqkv_d[i * 128:(i + 1) * 128, :])
                nc.sync.dma_start(wpT[i][:], wpT_d[i * 128:(i + 1) * 128, :])
                nc.sync.dma_start(wout[i][:], wout_d[i * 128:(i + 1) * 128, :])
                nc.sync.dma_start(gt[i][:], gt_d[i * 128:(i + 1) * 128, :])
                nc.sync.dma_start(posu[i][:], posu_d[i * 128:(i + 1) * 128, :])
                nc.sync.dma_start(posv[i][:], posv_d[i * 128:(i + 1) * 128, :])
                nc.sync.dma_start(bout[i][:], bout_d[i * 128:(i + 1) * 128, :])
            for i in range(2):
                nc.sync.dma_start(sing[i][:], sing_d[i * 128:(i + 1) * 128, :])
                nc.sync.dma_start(cosg[i][:], cosg_d[i * 128:(i + 1) * 128, :])
            nc.sync.dma_start(dmask[:], dmask_d[:, :])
            nc.sync.dma_start(dneg[:], dneg_d[:, :])
            nc.sync.dma_start(ltau[:], ltau_d[:, :])

            # m = exp(ltau) * DH^-0.5  (log-space fold of the 1/8 scale)
            nc.vector.memset(lnm[:], float(np.log(DH ** -0.5)))
            nc.scalar.activation(mb[:], ltau[:], AF.Exp, bias=lnm[:], scale=1.0)

            with (
                tc.tile_pool(name="ps_qkv", bufs=2, space="PSUM") as ps_qkv,
                tc.tile_pool(name="ps_p", bufs=2, space="PSUM") as ps_p,
                tc.tile_pool(name="ps_e", bufs=2, space="PSUM") as ps_e,
                tc.tile_pool(name="ps_av", bufs=2, space="PSUM") as ps_av,
                tc.tile_pool(name="work", bufs=2) as wk,
            ):
                # ---- QKV^T (feature-major Q,K) ------------------------
                for ntile in range(8):  # n in [0, 1024): Q and K parts
                    acc = ps_qkv.tile([128, T], f32, name="qkv_ps", tag="qkv")
                    for dt in range(ND):
                        nc.tensor.matmul(
                            acc[:],
                            wqkv[dt][:, ntile * 128:(ntile + 1) * 128],
                            xT[dt][:],
                            start=(dt == 0),
                            stop=(dt == ND - 1),
                        )
                    if ntile < 4:
                        nc.scalar.activation(
                            quT[ntile][:], acc[:], AF.Identity,
                            bias=posu[ntile][:], scale=1.0,
                        )
                        nc.scalar.activation(
                            qvT[ntile][:], acc[:], AF.Identity,
                            bias=posv[ntile][:], scale=1.0,
                        )
                    else:
                        nc.scalar.copy(kT[ntile - 4][:], acc[:])

                # ---- V token-major with ones column -------------------
                for it in range(NT):
                    acc = ps_qkv.tile([128, D], f32, name="v_ps", tag="qkv")
                    for dt in range(ND):
                        nc.tensor.matmul(
                            acc[:],
                            xT[dt][:, it * 128:(it + 1) * 128],
                            wqkv[dt][:, 2 * D:3 * D],
                            start=(dt == 0),
                            stop=(dt == ND - 1),
                        )
                    # copy (128, 8, 64) psum -> (128, 8, 128)[:, :, 0:64];
                    # cols 64:128 of each head block are ones so the AV
                    # matmul emits the softmax denominator on partitions
                    # 64:128 (replicated, ready for per-partition division).
                    vview = vsb[it][:].rearrange("p (h c) -> p h c", c=128)
                    nc.scalar.copy(
                        vview[:, :, 0:64],
                        acc[:].rearrange("p (h c) -> p h c", c=64),
                    )
                    nc.vector.memset(vview[:, :, 64:128], 1.0)

                # ---- per-head pipeline --------------------------------
                for h in range(H):
                    hd_tile = h // 2
                    hd_off = (h % 2) * 64
                    qv_h = qvT[hd_tile][hd_off:hd_off + 64, :]
                    qu_h = quT[hd_tile][hd_off:hd_off + 64, :]
                    k_h = kT[hd_tile]
                    wp_h = wpT[hd_tile]

                    # P^T[g, i] per g-tile, then modulate into C^T
                    ct = [
                        wk.tile([128, T], bf16, name=f"ct{g}", tag=f"ct{g}")
                        for g in range(ND)
                    ]
                    pt = [
                        wk.tile([128, T], bf16, name=f"pt{g}", tag=f"pt{g}")
                        for g in range(ND)
                    ]
                    for g in range(ND):
                        pacc = ps_p.tile([128, T], f32, name="p_ps", tag="p")
                        nc.tensor.matmul(
                            pacc[:],
                            wp_h[hd_off:hd_off + 64, g * 128:(g + 1) * 128]
                            if False else
                            wpT[hd_tile][hd_off:hd_off + 64, g * 128:(g + 1) * 128],
                            qv_h,
                            start=True,
                            stop=True,
                        )
                        nc.scalar.copy(pt[g][:], pacc[:])
                    # C1 = sin*Psin + cos*Pcos ; C2 = sin*Pcos - cos*Psin
                    for f in range(2):
                        a = wk.tile([128, T], bf16, name="moda", tag="moda")
                        bmod = wk.tile([128, T], bf16, name="modb", tag="modb")
                        nc.vector.tensor_mul(a[:], sing[f][:], pt[f][:])
                        nc.vector.tensor_mul(bmod[:], cosg[f][:], pt[2 + f][:])
                        nc.vector.tensor_add(ct[f][:], a[:], bmod[:])
                        a2 = wk.tile([128, T], bf16, name="moda2", tag="moda")
                        b2 = wk.tile([128, T], bf16, name="modb2", tag="modb")
                        nc.vector.tensor_mul(a2[:], sing[f][:], pt[2 + f][:])
                        nc.vector.tensor_mul(b2[:], cosg[f][:], pt[f][:])
                        nc.vector.tensor_sub(ct[2 + f][:], a2[:], b2[:])

                    # energy^T per j-tile: K-part + G-part + diag mask
                    attnT = [
                        wk.tile([128, T], bf16, name=f"attnT{j}", tag=f"attnT{j}")
                        for j in range(NT)
                    ]
                    for jt in range(NT):
                        eacc = ps_e.tile([128, T], f32, name="e_ps", tag="e")
                        nc.tensor.matmul(
                            eacc[:],
                            k_h[hd_off:hd_off + 64, jt * 128:(jt + 1) * 128],
                            qu_h,
                            start=True,
                            stop=False,
                        )
                        for g in range(ND):
                            nc.tensor.matmul(
                                eacc[:],
                                gt[g][:, jt * 128:(jt + 1) * 128],
                                ct[g][:],
                                start=False,
                                stop=False,
                            )
                        off = 384 - jt * 128
                        nc.tensor.matmul(
                            eacc[:],
                            dneg[:],
                            dmask[:, off:off + 512],
                            start=False,
                            stop=True,
                        )
                        nc.scalar.activation(
                            attnT[jt][:], eacc[:], AF.Exp, bias=0.0, scale=mb[:],
                        )

                    # AV^T; partitions 64:128 get the softmax denominator
                    avacc = ps_av.tile([128, T], f32, name="av_ps", tag="av")
                    for jt in range(NT):
                        nc.tensor.matmul(
                            avacc[:],
                            vsb[jt][:, h * 128:(h + 1) * 128],
                            attnT[jt][:],
                            start=(jt == 0),
                            stop=(jt == NT - 1),
                        )
                    rdb = wk.tile([64, T], f32, name="rdb", tag="rdb")
                    nc.vector.reciprocal(rdb[:], avacc[64:128, :])
                    nc.vector.tensor_mul(
                        avn[hd_tile][hd_off:hd_off + 64, :],
                        avacc[0:64, :],
                        rdb[:],
                    )

                # ---- output projection --------------------------------
                for ot in range(ND):
                    oacc = ps_p.tile([128, T], f32, name="o_ps", tag="p")
                    for dt in range(ND):
                        nc.tensor.matmul(
                            oacc[:],
                            wout[dt][:, ot * 128:(ot + 1) * 128],
                            avn[dt][:],
                            start=(dt == 0),
                            stop=(dt == ND - 1),
                        )
                    nc.scalar.activation(
                        outsb[ot][:], oacc[:], AF.Identity,
                        bias=bout[ot][:], scale=1.0,
                    )
                    nc.sync.dma_start(
                        out_d[ot * 128:(ot + 1) * 128, :], outsb[ot][:]
                    )

    _split_multi_waits(nc)
    return nc


def _host_constants():
    freqs = np.exp(
        -np.log(10000.0) * np.arange(HALF, dtype=np.float32) / HALF
    )
    idx = np.arange(T, dtype=np.float32)
    ang = np.outer(freqs, idx)  # (HALF, T)
    sing = np.sin(ang).astype(BF16)
    cosg = np.cos(ang).astype(BF16)
    gt = np.concatenate([np.cos(ang), np.sin(ang)], axis=0).astype(BF16)
    dmask = np.zeros((128, 896), dtype=BF16)
    dmask[np.arange(128), np.arange(128) + 384] = 1.0
    dneg = (NEG_BIG * np.eye(128, dtype=np.float32)).astype(BF16)
    return sing, cosg, gt, dmask, dneg


def kernel(x, W_qkv, W_pos, pos_u, pos_v, W_out, b_out, log_one_div_by_tau):
    from concourse import bass_utils

    if "nc" not in _CACHE:
        _CACHE["nc"] = _build()
        _CACHE["consts"] = _host_constants()
    nc = _CACHE["nc"]
    sing, cosg, gt, dmask, dneg = _CACHE["consts"]

    x = np.asarray(x, np.float32)
    shared = {
        "wqkv": np.ascontiguousarray(W_qkv, dtype=np.float32).astype(BF16),
        "wpT": np.ascontiguousarray(np.asarray(W_pos, np.float32).T).astype(BF16),
        "wout": np.ascontiguousarray(W_out, dtype=np.float32).astype(BF16),
        "posu": np.asarray(pos_u, np.float32).reshape(D, 1),
        "posv": np.asarray(pos_v, np.float32).reshape(D, 1),
        "bout": np.asarray(b_out, np.float32).reshape(D, 1),
        "ltau": np.full((128, 1), np.float32(np.asarray(log_one_div_by_tau).reshape(-1)[0])),
        "sing": sing, "cosg": cosg, "gt": gt,
        "dmask": dmask, "dneg": dneg,
    }
    in_maps = []
    for b in range(B):
        m = dict(shared)
        m["xT"] = np.ascontiguousarray(x[:, b, :].T).astype(BF16)
        in_maps.append(m)

    _CACHE["last_in_maps"] = in_maps
    res = bass_utils.run_bass_kernel_spmd(nc, in_maps, core_ids=list(range(B)))
    out = np.empty((T, B, D), np.float32)
    for b in range(B):
        out[:, b, :] = res.results[b]["out"].T
    return out
